# revision 1
# baseline (speedup 1.0000x reference)
"""Trainium2 Bass kernel for nn_DecoderBlock (self-attn + cross-attn + MLP), 8 cores.

Sharding: data-parallel over (batch, sequence-half): core c handles batch b=c//2
and query rows [h*1024,(h+1)*1024), h=c%2.  K/V for the full sequence of the
core's batch element are computed redundantly on both cores of the pair, so no
collectives are needed.  The host permutes x rows so each core's query rows are
always rows 0:1024 of its x input (attention is permutation-invariant over keys).

Device layouts (per core):
  residual stream: natural [tokens(part), C] fp32
  LN outputs:      transposed [C-chunk(part), 6, tokens] bf16 (PE transpose)
  Q^T/K^T:         [128 = head-pair dims(part), 6, tokens] bf16
  V:               [128 tokens(part), 16, 768] bf16
  scores:          S^T [keys(part), queries]; exp fused into PSUM->SBUF copyback
  softmax denom:   ones-column matmuls (PE col-tiling); normalize after AV via
                   DMA partition-broadcast + tensor_tensor multiply
LN gamma/beta are folded into downstream weights/biases on the host.
"""

import contextlib

import numpy as np
import ml_dtypes

import concourse.bass as bass
import concourse.mybir as mybir
import concourse.tile as tile
from concourse import bacc
from concourse.bass import ds, ts
from concourse.bass_utils import run_bass_kernel_spmd
from concourse.masks import make_identity

FP32 = mybir.dt.float32
BF16 = mybir.dt.bfloat16
AF = mybir.ActivationFunctionType
ALU = mybir.AluOpType

B, N, C, H = 4, 2048, 768, 12
D = C // H            # 64
HID = 4 * C           # 3072
NQ = N // 2           # 1024 queries per core
EPS = 1e-5
SCALE = float(D) ** -0.5
NCH = C // 128        # 6 contraction chunks over C
NT_ALL = N // 128     # 16
NT_Q = NQ // 128      # 8
NHP = H // 2          # 6 head pairs
NKC = N // 128        # 16 key chunks
USE_DMA_TRANSPOSE = True  # LN transposes via DMA xbar instead of PE


class _Prog:
    pass


def _build(P):
    nc = P.nc
    tc = P.tc

    # ---------- constant / persistent tiles ----------
    consts = P.consts
    identity = consts.tile([128, 128], BF16, tag="identity", name="identity")
    make_identity(nc, identity[:])
    ones_col = consts.tile([128, 1], BF16, tag="ones_col", name="ones_col")
    nc.vector.memset(ones_col[:], 1.0)
    ones_row = consts.tile([1, 128], BF16, tag="ones_row", name="ones_row")
    nc.vector.memset(ones_row[:], 1.0)
    ones128f = consts.tile([128, 64], FP32, tag="ones128f", name="ones128f")
    nc.vector.memset(ones128f[:], 1.0)
    eps_t = consts.tile([128, 1], FP32, tag="eps_t", name="eps_t")
    nc.vector.memset(eps_t[:], EPS)
    if P.with_bias:
        qkb = consts.tile([128, 12], FP32, tag="qkb", name="qkb")
        nc.sync.dma_start(qkb[:], P.qkb_d.rearrange("(j p) -> p j", p=128))
        qkb2 = consts.tile([128, 12], FP32, tag="qkb2", name="qkb2")
        nc.sync.dma_start(qkb2[:], P.qkb2_d.rearrange("(j p) -> p j", p=128))
        fc1b = consts.tile([128, 24], FP32, tag="fc1b", name="fc1b")
        nc.sync.dma_start(fc1b[:], P.fc1b_d.rearrange("(j p) -> p j", p=128))
        brows = []
        for i in range(5):
            r = consts.tile([1, C], BF16, tag=f"brow{i}", name=f"brow{i}")
            nc.sync.dma_start(r[:], P.brows_d[i:i + 1, :])
            brows.append(r)
        brow_v_sa, brow_o_sa, brow_v_ca, brow_o_ca, brow_fc2 = brows
    else:
        qkb = qkb2 = fc1b = None
        brow_v_sa = brow_o_sa = brow_v_ca = brow_o_ca = brow_fc2 = None
    P.identity, P.ones_col, P.ones_row = identity, ones_col, ones_row

    small = P.small

    # ---------- helpers ----------
    def rsqrt_dve(var_ap, rstd):
        """rstd = 1/sqrt(var+eps) computed entirely on DVE (Newton from a
        bit-hack seed) so LayerNorms never touch the ACT engine: ACT table-set
        swaps against attention's Exp cost ~2.7us each."""
        v = small.tile([128, 1], FP32, tag="rs_v", name="rs_v")
        nc.vector.tensor_scalar_add(v[:], var_ap, EPS)
        # y0 bits = 0x5F3759DF - (bits(v) >> 1)  ==  (~(bits(v)>>1)) + 0x5F3759E0
        yi = small.tile([128, 1], mybir.dt.int32, tag="rs_yi", name="rs_yi")
        nc.vector.tensor_scalar(yi[:], v[:].bitcast(mybir.dt.int32), 1, -1,
                                ALU.arith_shift_right, ALU.bitwise_xor)
        y = small.tile([128, 1], FP32, tag="rs_y", name="rs_y")
        nc.vector.tensor_scalar_add(y[:].bitcast(mybir.dt.int32), yi[:],
                                    0x5F3759E0)
        t1 = small.tile([128, 1], FP32, tag="rs_t1", name="rs_t1")
        t2 = small.tile([128, 1], FP32, tag="rs_t2", name="rs_t2")
        for _ in range(3):
            nc.vector.tensor_tensor(t1[:], y[:], y[:], ALU.mult)
            nc.vector.tensor_tensor(t2[:], t1[:], v[:], ALU.mult)
            nc.vector.tensor_scalar(t1[:], t2[:], -0.5, 1.5, ALU.mult, ALU.add)
            nc.vector.tensor_tensor(rstd[:], y[:], t1[:], ALU.mult)
            y, rstd = rstd, y
        return y  # after swap, y holds the final value

    def ln_transpose(x_tiles, lnt, ntc):
        """LayerNorm list of [128,768] fp32 tiles -> lnt [128, 6, ntc*128] bf16 (transposed)."""
        tpsum = P.tpsum
        for t in range(ntc):
            xt = x_tiles[t]
            st = small.tile([128, 2, 6], FP32, tag="ln_st", name="ln_st")
            nc.vector.bn_stats(st[:, 0, :], xt[:, 0:384])
            nc.vector.bn_stats(st[:, 1, :], xt[:, 384:768])
            mv = small.tile([128, 2], FP32, tag="ln_mv", name="ln_mv")
            nc.vector.bn_aggr(mv[:], st[:])
            rstd0 = small.tile([128, 1], FP32, tag="ln_rstd", name="ln_rstd")
            rstd = rsqrt_dve(mv[:, 1:2], rstd0)
            xn = small.tile([128, 768], BF16, tag="ln_xn", name="ln_xn")
            nc.vector.tensor_scalar(xn[:], xt[:], mv[:, 0:1], rstd[:],
                                    ALU.subtract, ALU.mult)
            if USE_DMA_TRANSPOSE:
                for ci in range(NCH):
                    nc.sync.dma_start_transpose(lnt[:, ci, ts(t, 128)],
                                                xn[:, ts(ci, 128)])
            else:
                for ci in range(NCH):
                    pt = tpsum.tile([128, 128], BF16, tag="tr", name="tr")
                    nc.tensor.transpose(pt[:], xn[:, ts(ci, 128)], identity[:])
                    nc.vector.tensor_copy(lnt[:, ci, ts(t, 128)], pt[:])

    def swapped_gemm(w_dram, col_off, nj, lnt, ntok, out_tt, bias_tile=None,
                     bias_off=0, act=None):
        """out_tt[:, j, :] ([128, nj, ntok] bf16) = (W[:, col_off:col_off+nj*128].T @ LN^T) + b."""
        with tc.tile_pool(name="swps", bufs=2, space="PSUM") as swps, \
                tc.tile_pool(name="w_big", bufs=1) as wpool:
            wsb = wpool.tile([128, NCH, nj * 128], BF16, tag="w_big", name="w_big")
            nc.sync.dma_start(
                wsb[:], w_dram[:, ds(col_off, nj * 128)].rearrange("(o p) n -> p o n", p=128))
            for j in range(nj):
                for tq2 in range(max(1, ntok // 1024)):
                    width = min(1024, ntok)
                    ps = swps.tile([128, 1024], FP32, tag="sw_ps", name="sw_ps")
                    for half in range(width // 512):
                        for ci in range(NCH):
                            nc.tensor.matmul(
                                ps[:, ds(half * 512, 512)], wsb[:, ci, ts(j, 128)],
                                lnt[:, ci, ds(tq2 * 1024 + half * 512, 512)],
                                start=(ci == 0), stop=(ci == NCH - 1))
                    dst = out_tt[:, j, ds(tq2 * 1024, width)]
                    src = ps[:, 0:width]
                    if act is not None:
                        if bias_tile is not None:
                            nc.scalar.activation(
                                dst, src, act,
                                bias=bias_tile[:, bias_off + j:bias_off + j + 1])
                        else:
                            nc.scalar.activation(dst, src, act)
                    elif bias_tile is not None:
                        nc.vector.tensor_scalar_add(
                            dst, src, bias_tile[:, bias_off + j:bias_off + j + 1])
                    else:
                        nc.vector.tensor_copy(dst, src)

    def normal_gemm(src_tt, nk, w_dram, bias_row, ntc, consumer):
        """psum[t] [128,768] = src^T[:, :, t].T @ W + bias_row; consumer(t, ps)."""
        with tc.tile_pool(name="natps", bufs=2, space="PSUM") as natps, \
                tc.tile_pool(name="w_nat", bufs=1) as wpool:
            wsb = wpool.tile([128, nk, 768], BF16, tag="w_nat", name="w_nat")
            nc.sync.dma_start(wsb[:], w_dram.rearrange("(o p) n -> p o n", p=128))
            for t in range(ntc):
                ps = natps.tile([128, 768], FP32, tag="nat_ps", name="nat_ps")
                for sl in (slice(0, 512), slice(512, 768)):
                    for ki in range(nk):
                        nc.tensor.matmul(ps[:, sl], src_tt[:, ki, ts(t, 128)],
                                         wsb[:, ki, sl], start=(ki == 0),
                                         stop=(bias_row is None and ki == nk - 1))
                    if bias_row is not None:
                        nc.tensor.matmul(ps[:, sl], ones_row[:], bias_row[:, sl],
                                         start=False, stop=True)
                consumer(t, ps)

    def attention(qt, kt, v, ot):
        """qt [128,6,1024], kt [128,6,2048], v [128,16,768] -> ot [128,6,1024] (normalized).

        Pipelined at key-chunk-pair granularity: the PE stream interleaves
        scores(e) with AV/denominator matmuls for e-1, so the PE stays busy
        while ACT streams the exps.  One PSUM accumulation group per bank.
        """
        with tc.tile_pool(name="scps", bufs=2, space="PSUM") as scps, \
                tc.tile_pool(name="avps", bufs=1, space="PSUM") as avps, \
                tc.tile_pool(name="dps", bufs=1, space="PSUM") as dps, \
                tc.tile_pool(name="atpool", bufs=8) as atpool:
            for hp in range(NHP):
                for tq in range(2):
                    qsl = ts(tq, 512)
                    po = avps.tile([128, 1024], FP32, tag="av_ps", name="av_ps")
                    pd = dps.tile([33, 1024], FP32, tag="d_ps", name="d_ps")
                    ats = {}

                    def scores(e, hp=hp, qsl=qsl, ats=ats):
                        for hh in range(2):
                            sc = scps.tile([128, 1024], FP32, tag="sc_ps", name="sc_ps")
                            for i in range(2):
                                kc = e * 2 + i
                                nc.tensor.matmul(
                                    sc[:, ds(i * 512, 512)],
                                    kt[ds(hh * 64, 64), hp, ts(kc, 128)],
                                    qt[ds(hh * 64, 64), hp, qsl],
                                    start=True, stop=True,
                                    tile_position=(hh * 64, 0))
                            a = atpool.tile([128, 2, 512], BF16, tag="at", name="at")
                            nc.scalar.activation(a[:], sc[:], AF.Exp, scale=SCALE)
                            ats[(hh, e)] = a

                    def av(e, hp=hp, po=po, pd=pd, ats=ats):
                        for i in range(2):
                            kc = e * 2 + i
                            first, last = kc == 0, kc == NKC - 1
                            for hh in range(2):
                                nc.tensor.matmul(
                                    po[ds(hh * 64, 64), ds(hh * 512, 512)],
                                    v[:, kc, ds((2 * hp + hh) * 64, 64)],
                                    ats[(hh, e)][:, i, :], start=first, stop=last,
                                    tile_position=(0, hh * 64))
                            nc.tensor.matmul(
                                pd[0:1, 0:512], ones_col[:], ats[(0, e)][:, i, :],
                                start=first, stop=last, tile_position=(0, 0))
                            nc.tensor.matmul(
                                pd[32:33, 512:1024], ones_col[:], ats[(1, e)][:, i, :],
                                start=first, stop=last, tile_position=(0, 32))

                    for e in range(8):
                        scores(e)
                        if e > 0:
                            av(e - 1)
                    av(7)
                    rc = small.tile([33, 1024], FP32, tag="drecip", name="drecip")
                    nc.vector.reciprocal(rc[0:1, 0:512], pd[0:1, 0:512])
                    nc.vector.reciprocal(rc[32:33, 512:1024], pd[32:33, 512:1024])
                    rbp = dps.tile([128, 1024], FP32, tag="d_ps", name="rbp")
                    nc.tensor.matmul(rbp[0:64, 0:512], ones128f[0:1, :],
                                     rc[0:1, 0:512], start=True, stop=True,
                                     tile_position=(0, 0))
                    nc.tensor.matmul(rbp[64:128, 512:1024], ones128f[32:33, :],
                                     rc[32:33, 512:1024], start=True, stop=True,
                                     tile_position=(32, 64))
                    osb = small.tile([128, 512], FP32, tag="osb", name="osb")
                    for hh in range(2):
                        sl = ds(hh * 512, 512)
                        nc.vector.tensor_copy(osb[ds(hh * 64, 64), :],
                                              po[ds(hh * 64, 64), sl])
                        nc.vector.tensor_tensor(ot[ds(hh * 64, 64), hp, qsl],
                                                osb[ds(hh * 64, 64), :],
                                                rbp[ds(hh * 64, 64), sl], ALU.mult)

    # ---------- phase 1: load x, LN1, transpose ----------
    xres = [P.xres.tile([128, 768], FP32, tag="xres", name="xres") for _ in range(NT_Q)]
    x_tiles = list(xres)
    for t in range(NT_ALL):
        if t < NT_Q:
            xt = xres[t]
        else:
            xt = small.tile([128, 768], FP32, tag="xtmp", name="xtmp")
            x_tiles.append(xt)
        nc.sync.dma_start(xt[:], P.x_d[ts(t, 128), :])

    ln1t = P.lnt_big.tile([128, NCH, N], BF16, tag="lnt_big", name="lnt_big")
    ln_transpose(x_tiles, ln1t, NT_ALL)

    with tc.tile_pool(name="qkv", bufs=1) as qkvp:
        # ---------- phase 2: self qkv ----------
        qt = qkvp.tile([128, NHP, NQ], BF16, tag="qt", name="qt")
        kt = qkvp.tile([128, NHP, N], BF16, tag="kt", name="kt")
        v = qkvp.tile([128, NKC, 768], BF16, tag="v", name="v")
        ot = qkvp.tile([128, NHP, NQ], BF16, tag="ot", name="ot")
        swapped_gemm(P.wqkv_d, 0, NHP, ln1t[:, :, 0:NQ], NQ, qt,
                     bias_tile=qkb, bias_off=0)
        swapped_gemm(P.wqkv_d, C, NHP, ln1t, N, kt, bias_tile=qkb, bias_off=6)
        normal_gemm(ln1t, NCH, P.wqkv_d[:, 2 * C:3 * C], brow_v_sa, NT_ALL,
                    lambda t, ps: nc.any.tensor_copy(v[:, t, :], ps[:]))

        # ---------- phase 3: self attention ----------
        attention(qt, kt, v, ot)

        # ---------- phase 4: self o-proj + residual ----------
        normal_gemm(ot, NCH, P.wo_sa_d, brow_o_sa, NT_Q,
                    lambda t, ps: nc.vector.tensor_tensor(
                        xres[t][:], ps[:], xres[t][:], ALU.add))

        # ---------- phase 5: LN(y), LN2(x) ----------
        y_tiles = []
        for t in range(NT_ALL):
            yt = small.tile([128, 768], FP32, tag="xtmp", name="xtmp")
            nc.sync.dma_start(yt[:], P.y_d[ts(t, 128), :])
            y_tiles.append(yt)
        lnyt = P.lnt_big.tile([128, NCH, N], BF16, tag="lnt_big", name="lnt_big")
        ln_transpose(y_tiles, lnyt, NT_ALL)
        ln2t = P.lnt_small.tile([128, NCH, NQ], BF16, tag="lnt_small", name="lnt_small")
        ln_transpose(xres, ln2t, NT_Q)

        # ---------- phase 6: cross qkv ----------
        qt2 = qkvp.tile([128, NHP, NQ], BF16, tag="qt", name="qt")
        kt2 = qkvp.tile([128, NHP, N], BF16, tag="kt", name="kt")
        v2 = qkvp.tile([128, NKC, 768], BF16, tag="v", name="v")
        ot2 = qkvp.tile([128, NHP, NQ], BF16, tag="ot", name="ot")
        swapped_gemm(P.wq_d, 0, NHP, ln2t, NQ, qt2, bias_tile=qkb2, bias_off=0)
        swapped_gemm(P.wk_d, 0, NHP, lnyt, N, kt2, bias_tile=qkb2, bias_off=6)
        normal_gemm(lnyt, NCH, P.wv_d, brow_v_ca, NT_ALL,
                    lambda t, ps: nc.any.tensor_copy(v2[:, t, :], ps[:]))

        # ---------- phase 7: cross attention ----------
        attention(qt2, kt2, v2, ot2)

        # ---------- phase 8: cross o-proj + residual ----------
        normal_gemm(ot2, NCH, P.wo_ca_d, brow_o_ca, NT_Q,
                    lambda t, ps: nc.vector.tensor_tensor(
                        xres[t][:], ps[:], xres[t][:], ALU.add))

    # ---------- phase 9: MLP ----------
    ln3t = P.lnt_small.tile([128, NCH, NQ], BF16, tag="lnt_small", name="lnt_small")
    ln_transpose(xres, ln3t, NT_Q)
    mlp = P.ctx.enter_context(tc.tile_pool(name="mlp", bufs=1))
    ht = mlp.tile([128, HID // 128, NQ], BF16, tag="ht", name="ht")
    swapped_gemm(P.w1_d, 0, 12, ln3t, NQ, ht[:, 0:12, :], bias_tile=fc1b,
                 bias_off=0, act=AF.Gelu)
    swapped_gemm(P.w1_d, 12 * 128, 12, ln3t, NQ, ht[:, 12:24, :], bias_tile=fc1b,
                 bias_off=12, act=AF.Gelu)

    def fc2_consumer(t, ps):
        ost = mlp.tile([128, 768], FP32, tag="ostage", name="ostage")
        nc.vector.tensor_tensor(ost[:], ps[:], xres[t][:], ALU.add)
        nc.sync.dma_start(P.out_d[ts(t, 128), :], ost[:])

    normal_gemm(ht, HID // 128, P.w2_d, brow_fc2, NT_Q, fc2_consumer)


def build_program(with_bias=True):
    P = _Prog()
    P.with_bias = with_bias
    nc = bacc.Bacc("TRN2", target_bir_lowering=False, debug=False, num_devices=8)
    P.nc = nc

    P.x_d = nc.dram_tensor("x", [N, C], FP32, kind="ExternalInput").ap()
    P.y_d = nc.dram_tensor("y", [N, C], FP32, kind="ExternalInput").ap()
    P.wqkv_d = nc.dram_tensor("wqkv", [C, 3 * C], BF16, kind="ExternalInput").ap()
    P.wo_sa_d = nc.dram_tensor("wo_sa", [C, C], BF16, kind="ExternalInput").ap()
    P.wq_d = nc.dram_tensor("wq", [C, C], BF16, kind="ExternalInput").ap()
    P.wk_d = nc.dram_tensor("wk", [C, C], BF16, kind="ExternalInput").ap()
    P.wv_d = nc.dram_tensor("wv", [C, C], BF16, kind="ExternalInput").ap()
    P.wo_ca_d = nc.dram_tensor("wo_ca", [C, C], BF16, kind="ExternalInput").ap()
    P.w1_d = nc.dram_tensor("w1", [C, HID], BF16, kind="ExternalInput").ap()
    P.w2_d = nc.dram_tensor("w2", [HID, C], BF16, kind="ExternalInput").ap()
    if with_bias:
        P.qkb_d = nc.dram_tensor("qkb", [2 * C], FP32, kind="ExternalInput").ap()
        P.qkb2_d = nc.dram_tensor("qkb2", [2 * C], FP32, kind="ExternalInput").ap()
        P.fc1b_d = nc.dram_tensor("fc1b", [HID], FP32, kind="ExternalInput").ap()
        P.brows_d = nc.dram_tensor("brows", [5, C], BF16, kind="ExternalInput").ap()
    P.out_d = nc.dram_tensor("out", [NQ, C], FP32, kind="ExternalOutput").ap()

    with tile.TileContext(nc) as tc:
        P.tc = tc
        with contextlib.ExitStack() as ctx:
            P.consts = ctx.enter_context(tc.tile_pool(name="consts", bufs=1))
            P.tpsum = None if USE_DMA_TRANSPOSE else ctx.enter_context(
                tc.tile_pool(name="tpsum", bufs=2, space="PSUM"))
            P.small = ctx.enter_context(tc.tile_pool(name="small", bufs=2))
            P.xres = ctx.enter_context(tc.tile_pool(name="xres", bufs=NT_Q))
            P.lnt_big = ctx.enter_context(tc.tile_pool(name="lnt_big", bufs=1))
            P.lnt_small = ctx.enter_context(tc.tile_pool(name="lnt_small", bufs=1))
            P.ctx = ctx
            _build(P)

    nc.compile()
    return nc


_NC = {}


def _needs_bias(g):
    vecs = [g['be1'] @ g['Wqkv'], g['be2'] @ g['Wq'], g['bey'] @ g['Wk'],
            g['bey'] @ g['Wv'], g['be3'] @ g['W1'] + g['b1'], g['bo_sa'],
            g['bo_ca'], g['b2']]
    return any(np.any(v != 0) for v in vecs)


def _prep_host(inputs, with_bias):
    f32 = np.float32
    g = {k: np.asarray(v, f32) for k, v in inputs.items()
         if k not in ('xpos', 'ypos', 'h', 'w')}
    bf = ml_dtypes.bfloat16

    wqkv = g['g1'][:, None] * g['Wqkv']
    wq = g['g2'][:, None] * g['Wq']
    wk = g['gy'][:, None] * g['Wk']
    wv = g['gy'][:, None] * g['Wv']
    w1 = g['g3'][:, None] * g['W1']

    shared = {
        'wqkv': wqkv.astype(bf),
        'wo_sa': g['Wo_sa'].astype(bf),
        'wq': wq.astype(bf),
        'wk': wk.astype(bf),
        'wv': wv.astype(bf),
        'wo_ca': g['Wo_ca'].astype(bf),
        'w1': w1.astype(bf),
        'w2': g['W2'].astype(bf),
    }
    if with_bias:
        bqkv = g['be1'] @ g['Wqkv']
        bq = g['be2'] @ g['Wq']
        bk = g['bey'] @ g['Wk']
        bv = g['bey'] @ g['Wv']
        bfc1 = g['be3'] @ g['W1'] + g['b1']
        shared.update({
            'qkb': np.concatenate([bqkv[0:C], bqkv[C:2 * C]]).astype(f32),
            'qkb2': np.concatenate([bq, bk]).astype(f32),
            'fc1b': bfc1.astype(f32),
            'brows': np.stack([bqkv[2 * C:3 * C], g['bo_sa'], bv, g['bo_ca'],
                               g['b2']]).astype(bf),
        })
    x = g['x']
    y = g['y']
    in_maps = []
    for c in range(8):
        b, hh = c // 2, c % 2
        xp = np.concatenate([x[b, hh * NQ:(hh + 1) * NQ],
                             x[b, (1 - hh) * NQ:(2 - hh) * NQ]], axis=0)
        in_maps.append({'x': np.ascontiguousarray(xp), 'y': np.ascontiguousarray(y[b]),
                        **shared})
    return in_maps


def kernel(**inputs):
    g = {k: np.asarray(v, np.float32) for k, v in inputs.items()
         if k not in ('xpos', 'ypos', 'h', 'w', 'x', 'y')}
    with_bias = _needs_bias(g)
    if with_bias not in _NC:
        _NC[with_bias] = build_program(with_bias)
    nc = _NC[with_bias]
    in_maps = _prep_host(inputs, with_bias)
    res = run_bass_kernel_spmd(nc, in_maps, core_ids=list(range(8)))
    out = np.empty((B, N, C), np.float32)
    for c in range(8):
        b, hh = c // 2, c % 2
        out[b, hh * NQ:(hh + 1) * NQ] = res.results[c]['out']
    return out



# revision 3
# speedup vs baseline: 1.4303x; 1.4303x over previous
"""Trainium2 Bass kernel for nn_DecoderBlock (self-attn + cross-attn + MLP), 8 cores.

Sharding: data-parallel over (batch, sequence-half): core c handles batch b=c//2
and query rows [h*1024,(h+1)*1024); host permutes x rows so the core's query
rows are rows 0:1024.  K/V computed redundantly per pair (no collectives).

Key structure vs the original baseline:
  - Softmax denominator folded into AV: ones column appended to the V
    stationary, AV runs as fp8 DoubleRow (stationary [128,2,80] = 2 key-chunk
    k-tiles x (64 dims + ones + pad), moving ats [128,2,512] fp8).
  - Q/K/ats in fp8e4: scores at bf16 rate, AV at DoubleRow rate.
  - Weights host-pretiled to [128, K/128, O]: contiguous weight DMAs.
  - LN transposes fused: one 3D dma_start_transpose per [128,768] tile.
  - Software pipelining: independent GEMM chunks (cross K/V/Q, o-proj, fc1)
    are pumped into the PE stream between attention iterations so the PE
    stays busy while ACT streams the exps.
  - gelu batched after all attention exps (ACT table thrash avoidance).
  - SBUF pools managed as two LIFO stacks (left/right sides) by lifetime era.
"""

import contextlib

import numpy as np
import ml_dtypes

import concourse.bass as bass
import concourse.mybir as mybir
import concourse.tile as tile
from concourse import bacc
from concourse.bass import ds, ts
from concourse.bass_utils import run_bass_kernel_spmd

FP32 = mybir.dt.float32
BF16 = mybir.dt.bfloat16
FP8 = mybir.dt.float8e4
AF = mybir.ActivationFunctionType
ALU = mybir.AluOpType
DR = mybir.MatmulPerfMode.DoubleRow

B, N, C, H = 4, 2048, 768, 12
D = C // H            # 64
HID = 4 * C           # 3072
NQ = N // 2           # 1024 queries per core
EPS = 1e-5
SCALE = float(D) ** -0.5
NCH = C // 128        # 6
NT_ALL = N // 128     # 16
NT_Q = NQ // 128      # 8
NHP = H // 2          # 6


class _Prog:
    pass


def _build(P):
    nc = P.nc
    tc = P.tc
    ctx = P.ctx

    # --------- persistent pools (left stack bottom) ---------
    consts = ctx.enter_context(tc.tile_pool(name="consts", bufs=1))
    small = ctx.enter_context(tc.tile_pool(name="small", bufs=2))
    xresp = ctx.enter_context(tc.tile_pool(name="xres", bufs=NT_Q))
    lnbig = ctx.enter_context(tc.tile_pool(name="lnbig", bufs=1))
    gemmps = ctx.enter_context(tc.tile_pool(name="gemmps", bufs=1, space="PSUM"))
    # xov: xn staging (x2), attention output, V-with-ones; era1 -> post-cross
    xov = tc.alloc_tile_pool(name="xov", bufs=1)
    # selfp: self-era weights, y staging, q/k, ln2; era1 -> post-self
    selfp = tc.alloc_tile_pool(name="selfp", bufs=1)

    ones_row = consts.tile([1, 128], BF16, tag="ones_row", name="ones_row")
    nc.vector.memset(ones_row[:], 1.0)
    if P.with_bias:
        qkb = consts.tile([128, 12], FP32, tag="qkb", name="qkb")
        nc.sync.dma_start(qkb[:], P.qkb_d.rearrange("(j p) -> p j", p=128))
        qkb2 = consts.tile([128, 12], FP32, tag="qkb2", name="qkb2")
        nc.sync.dma_start(qkb2[:], P.qkb2_d.rearrange("(j p) -> p j", p=128))
        fc1b = consts.tile([128, 24], FP32, tag="fc1b", name="fc1b")
        nc.sync.dma_start(fc1b[:], P.fc1b_d.rearrange("(j p) -> p j", p=128))
        brows = []
        for i in range(5):
            r = consts.tile([1, C], BF16, tag=f"brow{i}", name=f"brow{i}")
            nc.sync.dma_start(r[:], P.brows_d[i:i + 1, :])
            brows.append(r)
        brow_v_sa, brow_o_sa, brow_v_ca, brow_o_ca, brow_fc2 = brows
    else:
        qkb = qkb2 = fc1b = None
        brow_v_sa = brow_o_sa = brow_v_ca = brow_o_ca = brow_fc2 = None

    # ---------- helpers ----------
    def rsqrt_dve(var_ap, rstd):
        v = small.tile([128, 1], FP32, tag="rs_v", name="rs_v")
        nc.vector.tensor_scalar_add(v[:], var_ap, EPS)
        yi = small.tile([128, 1], mybir.dt.int32, tag="rs_yi", name="rs_yi")
        nc.vector.tensor_scalar(yi[:], v[:].bitcast(mybir.dt.int32), 1, -1,
                                ALU.arith_shift_right, ALU.bitwise_xor)
        y = small.tile([128, 1], FP32, tag="rs_y", name="rs_y")
        nc.vector.tensor_scalar_add(y[:].bitcast(mybir.dt.int32), yi[:],
                                    0x5F3759E0)
        t1 = small.tile([128, 1], FP32, tag="rs_t1", name="rs_t1")
        t2 = small.tile([128, 1], FP32, tag="rs_t2", name="rs_t2")
        for _ in range(3):
            nc.vector.tensor_tensor(t1[:], y[:], y[:], ALU.mult)
            nc.vector.tensor_tensor(t2[:], t1[:], v[:], ALU.mult)
            nc.vector.tensor_scalar(t1[:], t2[:], -0.5, 1.5, ALU.mult, ALU.add)
            nc.vector.tensor_tensor(rstd[:], y[:], t1[:], ALU.mult)
            y, rstd = rstd, y
        return y

    P.xn_i = 0

    def ln_tile(xt, lnt_slice):
        """LN one [128,768] fp32 tile -> bf16, fused 3D transpose into
        lnt_slice [128, 6, 128]."""
        st = small.tile([128, 2, 6], FP32, tag="ln_st", name="ln_st")
        nc.vector.bn_stats(st[:, 0, :], xt[:, 0:384])
        nc.vector.bn_stats(st[:, 1, :], xt[:, 384:768])
        mv = small.tile([128, 2], FP32, tag="ln_mv", name="ln_mv")
        nc.vector.bn_aggr(mv[:], st[:])
        rstd0 = small.tile([128, 1], FP32, tag="ln_rstd", name="ln_rstd")
        rstd = rsqrt_dve(mv[:, 1:2], rstd0)
        xn = xov.tile([128, 768], BF16, tag=f"xn{P.xn_i % 2}", name="ln_xn")
        P.xn_i += 1
        nc.vector.tensor_scalar(xn[:], xt[:], mv[:, 0:1], rstd[:],
                                ALU.subtract, ALU.mult)
        nc.sync.dma_start_transpose(lnt_slice, xn[:])

    def qk_chunk(wsb, col_j, lnt, tok0, ntok, out_tt, out_j, bias_tile,
                 bias_off, dst_tok0=None):
        """One j-chunk of a weight-stationary GEMM:
        out_tt[:, out_j, dst_tok0:+ntok] = (W 128-col-chunk).T @ LN^T (+b)."""
        if dst_tok0 is None:
            dst_tok0 = tok0
        ps = P.cur_ps.tile([128, 1024], FP32, tag="gps", name="gps")
        for ci in range(NCH):
            for half in range(ntok // 512):
                nc.tensor.matmul(
                    ps[:, ds(half * 512, 512)], wsb[:, ci, ts(col_j, 128)],
                    lnt[:, ci, ds(tok0 + half * 512, 512)],
                    start=(ci == 0), stop=(ci == NCH - 1))
        dst = out_tt[:, out_j, ds(dst_tok0, ntok)]
        src = ps[:, 0:ntok]
        if bias_tile is not None:
            nc.vector.tensor_scalar_add(
                dst, src, bias_tile[:, bias_off + out_j:bias_off + out_j + 1])
        else:
            nc.any.tensor_copy(dst, src)

    def nat_chunk(src_tt, ki_list, wsb, wk0, bias_row, t, consumer,
                  psum_acc=None, first=True, last=True):
        """One 128-token chunk of a natural GEMM:
        ps[128,768] (+)= sum_ki src_tt[:,ki,t*128:].T @ wsb[:,wk0+n,:] (+bias)."""
        ps = psum_acc if psum_acc is not None else P.cur_ps.tile(
            [128, 1024], FP32, tag="gps", name="gps")
        for sl in (slice(0, 512), slice(512, 768)):
            for n, ki in enumerate(ki_list):
                nc.tensor.matmul(ps[:, sl], src_tt[:, ki, ts(t, 128)],
                                 wsb[:, wk0 + n, sl],
                                 start=(first and n == 0),
                                 stop=(last and bias_row is None
                                       and n == len(ki_list) - 1))
            if last and bias_row is not None:
                nc.tensor.matmul(ps[:, sl], ones_row[0:1, :], bias_row[:, sl],
                                 start=False, stop=True)
        if last:
            consumer(t, ps)
        return ps

    # ---------- pump machinery ----------
    queue = []

    def pump(budget_us):
        while queue and budget_us > 0:
            est, fn = queue.pop(0)
            fn()
            budget_us -= est

    # ---------- one attention iteration ----------
    P.at_i = 0
    P.pending_norm = None

    def flush_norm():
        """Emit the deferred normalize of the previous attention iteration.
        Deferring it past the next iteration's first scores keeps the PE from
        stalling on the recip->broadcast->mult chain at iteration boundaries."""
        if P.pending_norm is None:
            return
        pos, ot, hp, qsl = P.pending_norm
        P.pending_norm = None
        rbp = P.scps.tile([128, 2, 512], FP32, tag="sc", name="rbp")
        for hh in range(2):
            rc = small.tile([1, 512], BF16, tag="rc", name="rc")
            with nc.allow_low_precision(reason="softmax denom recip"):
                nc.vector.reciprocal(rc[:], pos[hh][64:65, :])
            nc.tensor.matmul(rbp[0:64, hh, :], ones_row[0:1, 0:64], rc[:],
                             start=True, stop=True)
            # DVE reads at most one PSUM operand: stage po rows via ACT copy
            osb = small.tile([64, 512], FP32, tag="osb", name="osb")
            nc.scalar.copy(osb[:], pos[hh][0:64, :])
            nc.vector.tensor_tensor(ot[ds(hh * 64, 64), hp, qsl],
                                    osb[:], rbp[0:64, hh, :], ALU.mult)

    def attention_iter(tq, hp, qt, kt, vv, ot, per_e=0.0, post=7.0):
        qsl = ts(tq, 512)
        pos = [None, None]
        ats = {}

        def scores(e):
            for hh in range(2):
                sc = P.scps.tile([128, 2, 512], FP32, tag="sc", name="sc")
                for i in range(2):
                    kc = e * 2 + i
                    nc.tensor.matmul(
                        sc[:, i, :], kt[ds(hh * 64, 64), hp, ts(kc, 128)],
                        qt[ds(hh * 64, 64), hp, qsl], start=True, stop=True,
                        tile_position=(hh * 64, 0))
                if hh == 0:
                    a = P.crossp.tile([128, 2, 512], FP8,
                                      tag=f"at{P.at_i % 4}", name="at")
                    nc.scalar.activation(a[:], sc[:], AF.Exp, scale=SCALE)
                    ats[(hh, e)] = a[:]
                else:
                    # DVE Schraudolph: fp8e4 bits ~= 8*log2(exp(s*SCALE)) + 56
                    a = P.crossp.tile([128, 2, 512], mybir.dt.int8,
                                      tag=f"at{P.at_i % 4}", name="at")
                    nc.vector.tensor_scalar(
                        a[:], sc[:], 8 * 1.4426950408889634 * SCALE, 56.0,
                        ALU.mult, ALU.add)
                    ats[(hh, e)] = a[:].bitcast(FP8)
                P.at_i += 1

        def av(e):
            for hh in range(2):
                nc.tensor.matmul(
                    pos[hh][:], vv[:, e, 2 * hp + hh, :, :], ats[(hh, e)],
                    start=(e == 0), stop=(e == 7), perf_mode=DR)

        for e in range(8):
            scores(e)
            if e == 0:
                flush_norm()
                pos[0] = P.avpo.tile([80, 512], FP32, tag="po", name="po")
                pos[1] = P.avpo.tile([80, 512], FP32, tag="po", name="po")
            if per_e > 0:
                pump(per_e)
            if e > 0:
                av(e - 1)
        av(7)
        P.pending_norm = (pos, ot, hp, qsl)
        if post > 0:
            pump(post)

    # ================= emission =================

    # ---------- era 1: loads, LN1, self QKV, LN(y) ----------
    xres = [xresp.tile([128, 768], FP32, tag="xres", name="xres")
            for _ in range(NT_Q)]

    with tc.tile_pool(name="xtmp", bufs=4) as xtmpp, \
            tc.tile_pool(name="wqkvp", bufs=1) as wqkvp, \
            tc.tile_pool(name="qkvps", bufs=2, space="PSUM") as qkvps:
        P.cur_ps = qkvps
        nc.sync.dma_start(xres[0][:], P.x_d[ts(0, 128), :])
        # wqkv: V-columns first so the V gemm (which only needs one LN tile
        # per chunk) can start as early as possible.
        wqkv_sb = wqkvp.tile([128, NCH, 3 * C], BF16, tag="wqkv", name="wqkv")
        nc.sync.dma_start(wqkv_sb[:, :, 2 * C:3 * C], P.wqkv_d[:, :, 2 * C:3 * C])
        for t in range(1, NT_Q):
            nc.sync.dma_start(xres[t][:], P.x_d[ts(t, 128), :])
        nc.sync.dma_start(wqkv_sb[:, :, 0:2 * C], P.wqkv_d[:, :, 0:2 * C])
        wk_sb = selfp.tile([128, NCH, C], BF16, tag="wk", name="wk_sb")
        nc.sync.dma_start(wk_sb[:], P.wk_d)
        wosa_sb = selfp.tile([128, NCH, C], BF16, tag="wosa", name="wosa_sb")
        nc.sync.dma_start(wosa_sb[:], P.wo_sa_d)
        wq_sb = selfp.tile([128, NCH, C], BF16, tag="wq", name="wq_sb")
        nc.sync.dma_start(wq_sb[:], P.wq_d)

        qt = selfp.tile([128, NHP, NQ], FP8, tag="qt", name="qt")
        kt = selfp.tile([128, NHP, N], FP8, tag="kt", name="kt")
        vv = xov.tile([128, 8, H, 2, 80], FP8, tag="vv", name="vv")
        ot = xov.tile([128, NHP, NQ], BF16, tag="ot", name="ot")

        def fill_v_chunk(vv_t, wsb, lnt, brow, t, eng=None):
            def vcons(t, ps):
                (eng or nc.any).tensor_copy(
                    vv_t[:, t // 2, :, t % 2, 0:64],
                    ps[:, 0:768].rearrange("p (h d) -> p h d", h=H))
            nat_chunk(lnt, list(range(NCH)), wsb, 0, brow, t, vcons)

        nc.vector.memset(vv[:, :, :, :, 64:80], 0.0)
        nc.vector.memset(vv[:, :, :, :, 64:65], 1.0)

        # LN1 (x) interleaved with V chunks (V chunk t needs only LN tile t)
        ln1t = lnbig.tile([128, NCH, N], BF16, tag="lnbig", name="ln1t")
        wqkv_v = wqkv_sb[:, :, 2 * C:3 * C]
        for t in range(NT_Q):
            ln_tile(xres[t], ln1t[:, 0:NCH, ts(t, 128)])
            fill_v_chunk(vv, wqkv_v, ln1t, brow_v_sa, t)
        for t in range(NT_Q, NT_ALL):
            xt = xtmpp.tile([128, 768], FP32, tag="xtmp", name="xtmp")
            nc.sync.dma_start(xt[:], P.x_d[ts(t, 128), :])
            ln_tile(xt, ln1t[:, 0:NCH, ts(t, 128)])
            fill_v_chunk(vv, wqkv_v, ln1t, brow_v_sa, t)

        # self Q then K
        for j in range(NHP):
            qk_chunk(wqkv_sb, j, ln1t, 0, 1024, qt, j, qkb, 0)
        for j in range(NHP):
            qk_chunk(wqkv_sb, NHP + j, ln1t, 0, 1024, kt, j, qkb, 6)
            qk_chunk(wqkv_sb, NHP + j, ln1t, 1024, 1024, kt, j, qkb, 6)

        # y loads on the ACT hwdge queue + LN(y) -> lnyt (reuses ln1t's slot,
        # so y-LN starts once self-QKV has finished reading ln1t).
        lnyt = lnbig.tile([128, NCH, N], BF16, tag="lnbig", name="lnyt")
        for t in range(NT_ALL):
            yt = selfp.tile([128, 768], FP32, tag=f"yst{t % 2}", name="yst")
            nc.scalar.dma_start(yt[:], P.y_d[ts(t, 128), :])
            ln_tile(yt, lnyt[:, 0:NCH, ts(t, 128)])

    P.cur_ps = gemmps

    # ---------- era 2: self attention + pumped crossK/o_sa/ln2/crossQ ----------
    # right-stack pool: cross q/k + ats (lives to end)
    P.crossp = tc.alloc_tile_pool(name="crossp", bufs=1, side="right")
    P.scps = tc.alloc_tile_pool(name="scps", bufs=2, space="PSUM")
    P.avpo = tc.alloc_tile_pool(name="avpo", bufs=2, space="PSUM")

    ln2t = selfp.tile([128, NCH, NQ], BF16, tag="ln2t", name="ln2t")
    qt2 = P.crossp.tile([128, NHP, NQ], FP8, tag="qt2", name="qt2")
    kt2 = P.crossp.tile([128, NHP, N], FP8, tag="kt2", name="kt2")

    for j in range(NHP):
        def ck(j=j):
            qk_chunk(wk_sb, j, lnyt, 0, 1024, kt2, j, qkb2, 6)
            qk_chunk(wk_sb, j, lnyt, 1024, 1024, kt2, j, qkb2, 6)
        queue.append((5.2, ck))

    def osa_chunk(t):
        def cons(t, ps):
            nc.vector.tensor_tensor(xres[t][:], ps[:, 0:768], xres[t][:],
                                    ALU.add)
        nat_chunk(ot, list(range(NHP)), wosa_sb, 0, brow_o_sa, t, cons)

    for tq in range(2):
        for hp in range(NHP):
            attention_iter(tq, hp, qt, kt, vv, ot)
        for t in range(tq * 4, tq * 4 + 4):
            queue.append((2.3, lambda t=t: osa_chunk(t)))
            queue.append((0.3, lambda t=t: ln_tile(
                xres[t], ln2t[:, 0:NCH, ts(t, 128)])))
        for j in range(NHP):
            queue.append((1.4, lambda j=j, tq=tq: qk_chunk(
                wq_sb, j, ln2t, tq * 512, 512, qt2, j, qkb2, 0)))
    # NOTE: the tq1 leftovers (o_sa, ln2, crossQ) stay queued; they drain
    # during cross-attn tq0 so cross scores/exps start immediately.

    # ---------- era 3: cross attention + pumped crossV/o_ca/ln3/fc1(tq0) ----
    w9b = tc.alloc_tile_pool(name="w9b", bufs=1, side="right")
    wv_sb = w9b.tile([128, NCH, C], BF16, tag="wv", name="wv_sb")
    nc.sync.dma_start(wv_sb[:], P.wv_d)
    woca_sb = w9b.tile([128, NCH, C], BF16, tag="woca", name="woca_sb")
    nc.sync.dma_start(woca_sb[:], P.wo_ca_d)

    vv2 = xov.tile([128, 8, H, 2, 80], FP8, tag="vv", name="vv2")
    nc.vector.memset(vv2[:, :, :, :, 64:80], 0.0)
    nc.vector.memset(vv2[:, :, :, :, 64:65], 1.0)
    # crossV goes to the FRONT of the queue: cross AV consumes it chunk by
    # chunk starting at (tq0, hp0).
    queue[0:0] = [(2.6, (lambda t=t: fill_v_chunk(
        vv2, wv_sb, lnyt, brow_v_ca, t, eng=nc.vector)))
        for t in range(NT_ALL)]

    ot2 = xov.tile([128, NHP, NQ], BF16, tag="ot", name="ot2")
    ln3t = lnbig.tile([128, NCH, NQ], BF16, tag="lnbig", name="ln3t")

    def oca_chunk(t):
        def cons(t, ps):
            nc.vector.tensor_tensor(xres[t][:], ps[:, 0:768], xres[t][:],
                                    ALU.add)
        nat_chunk(ot2, list(range(NHP)), woca_sb, 0, brow_o_ca, t, cons)

    w1h = [None, None]
    hpre = [None]

    def fc1_chunk(h, jloc, tq):
        jglob = h * 12 + jloc
        qk_chunk(w1h[h], jloc, ln3t, tq * 512, 512, hpre[0], jglob, fc1b, 0,
                 dst_tok0=0)

    hwsp = None
    for tq in range(2):
        for hp in range(NHP):
            per_e = 6.0 if (tq == 0 and hp <= 1) else (
                3.0 if (tq == 0 and hp == 2) else 0.0)
            attention_iter(tq, hp, qt2, kt2, vv2, ot2, per_e=per_e)
            if tq == 0 and hp == 2:
                # self-era leftovers and crossV are fully drained by now;
                # release selfp before hwsp pushes (SBUF headroom).
                pump(1e9)
                selfp.release()
        for t in range(tq * 4, tq * 4 + 4):
            queue.append((2.3, lambda t=t: oca_chunk(t)))
            queue.append((0.3, lambda t=t: ln_tile(
                xres[t], ln3t[:, 0:NCH, ts(t, 128)])))
        if tq == 0:
            # right-stack pool: hpre (per-tq) + weight halves (w1 then w2)
            hwsp = tc.alloc_tile_pool(name="hwsp", bufs=1, side="right")
            for h in range(2):
                w1h[h] = hwsp.tile([128, NCH, HID // 2], BF16, tag=f"ws{h}",
                                   name=f"w1h{h}")
                nc.sync.dma_start(w1h[h][:], P.w1_d[:, :, ds(h * 1536, 1536)])
            hpre[0] = hwsp.tile([128, 24, 512], BF16, tag="hp", name="hpre0")
            for h in range(2):
                for jloc in range(12):
                    queue.append((1.4, lambda h=h, j=jloc: fc1_chunk(h, j, 0)))

    # post-cross: drain (o_ca tq1, ln3 tq1, fc1 tq0 leftovers) on the tail psum
    P.avpo.release()
    P.scps.release()
    tailps = tc.alloc_tile_pool(name="tailps", bufs=2, space="PSUM")
    P.cur_ps = tailps
    pump(1e9)

    xov.release()

    # ---------- era 4: gelu(tq0); fc1(tq1); w2; fc2(tq0); gelu+fc2(tq1) ----
    with tc.tile_pool(name="htp", bufs=1) as htp, \
            tc.tile_pool(name="ostp", bufs=2) as ostp:
        w2h = [None, None]

        def gelu_tq(dst_ht, src_hpre):
            for t in range(4):
                nc.scalar.activation(dst_ht[:, :, ts(t, 128)],
                                     src_hpre[:, :, ts(t, 128)], AF.Gelu)

        def fc2_tq(tq, ht):
            for t in range(4):
                tg = tq * 4 + t
                ps = nat_chunk(ht, list(range(12)), w2h[0], 0, None, t, None,
                               first=True, last=False)

                def cons(_t, ps, tg=tg):
                    ost = ostp.tile([128, 768], FP32, tag="ost", name="ost")
                    nc.vector.tensor_tensor(ost[:], ps[:, 0:768], xres[tg][:],
                                            ALU.add)
                    nc.sync.dma_start(P.out_d[ts(tg, 128), :], ost[:])
                nat_chunk(ht, list(range(12, 24)), w2h[1], 0, brow_fc2, t,
                          cons, psum_acc=ps, first=False, last=True)

        ht0 = htp.tile([128, 24, 512], BF16, tag="ht", name="ht0")
        gelu_tq(ht0, hpre[0])
        hpre[0] = hwsp.tile([128, 24, 512], BF16, tag="hp", name="hpre1")
        for h in range(2):
            for jloc in range(12):
                fc1_chunk(h, jloc, 1)
        for h in range(2):
            w2h[h] = hwsp.tile([128, 12, C], BF16, tag=f"ws{h}", name=f"w2h{h}")
            nc.sync.dma_start(w2h[h][:], P.w2_d[:, ds(h * 12, 12), :])
        fc2_tq(0, ht0)
        ht1 = htp.tile([128, 24, 512], BF16, tag="ht", name="ht1")
        gelu_tq(ht1, hpre[0])
        fc2_tq(1, ht1)

    hwsp.release()
    w9b.release()
    P.crossp.release()
    tailps.release()


# revision 7
# speedup vs baseline: 1.6167x; 1.1303x over previous
"""Trainium2 Bass kernel for nn_DecoderBlock (self-attn + cross-attn + MLP), 8 cores.

Sharding: data-parallel over (batch, sequence-half): core c handles batch b=c//2
and query rows [h*1024,(h+1)*1024); host permutes x rows so the core's query
rows are rows 0:1024.  K/V computed redundantly per pair (no collectives).

Key structure vs the original baseline:
  - Softmax denominator folded into AV: ones column appended to the V
    stationary, AV runs as fp8 DoubleRow (stationary [128,2,80] = 2 key-chunk
    k-tiles x (64 dims + ones + pad), moving ats [128,2,512] fp8).
  - Q/K/ats in fp8e4: scores at bf16 rate, AV at DoubleRow rate.
  - Weights host-pretiled to [128, K/128, O]: contiguous weight DMAs.
  - LN transposes fused: one 3D dma_start_transpose per [128,768] tile.
  - Software pipelining: independent GEMM chunks (cross K/V/Q, o-proj, fc1)
    are pumped into the PE stream between attention iterations so the PE
    stays busy while ACT streams the exps.
  - gelu batched after all attention exps (ACT table thrash avoidance).
  - SBUF pools managed as two LIFO stacks (left/right sides) by lifetime era.
"""

import contextlib

import numpy as np
import ml_dtypes

import concourse.bass as bass
import concourse.mybir as mybir
import concourse.tile as tile
from concourse import bacc
from concourse.bass import ds, ts
from concourse.bass_utils import run_bass_kernel_spmd

FP32 = mybir.dt.float32
BF16 = mybir.dt.bfloat16
FP8 = mybir.dt.float8e4
AF = mybir.ActivationFunctionType
ALU = mybir.AluOpType
DR = mybir.MatmulPerfMode.DoubleRow

B, N, C, H = 4, 2048, 768, 12
D = C // H            # 64
HID = 4 * C           # 3072
NQ = N // 2           # 1024 queries per core
EPS = 1e-5
SCALE = float(D) ** -0.5
NCH = C // 128        # 6
NT_ALL = N // 128     # 16
NT_Q = NQ // 128      # 8
NHP = H // 2          # 6
WS = 256.0            # host-side weight scale (fp8 denormal avoidance)
RWS = 1.0 / WS


class _Prog:
    pass


def _build(P):
    nc = P.nc
    tc = P.tc
    ctx = P.ctx

    # --------- persistent pools (left stack bottom) ---------
    consts = ctx.enter_context(tc.tile_pool(name="consts", bufs=1))
    small = ctx.enter_context(tc.tile_pool(name="small", bufs=2))
    xresp = ctx.enter_context(tc.tile_pool(name="xres", bufs=NT_Q))
    lnbig = ctx.enter_context(tc.tile_pool(name="lnbig", bufs=1))
    gemmps = ctx.enter_context(tc.tile_pool(name="gemmps", bufs=1, space="PSUM"))
    # xov: xn staging (x2), attention output, V-with-ones; era1 -> post-cross
    xov = tc.alloc_tile_pool(name="xov", bufs=1)
    # selfp: self-era weights, y staging, q/k, ln2; era1 -> post-self
    selfp = tc.alloc_tile_pool(name="selfp", bufs=1)

    ones_row = consts.tile([1, 128], BF16, tag="ones_row", name="ones_row")
    nc.vector.memset(ones_row[:], 1.0)
    if P.with_bias:
        qkb = consts.tile([128, 12], FP32, tag="qkb", name="qkb")
        nc.sync.dma_start(qkb[:], P.qkb_d.rearrange("(j p) -> p j", p=128))
        qkb2 = consts.tile([128, 12], FP32, tag="qkb2", name="qkb2")
        nc.sync.dma_start(qkb2[:], P.qkb2_d.rearrange("(j p) -> p j", p=128))
        fc1b = consts.tile([128, 24], FP32, tag="fc1b", name="fc1b")
        nc.sync.dma_start(fc1b[:], P.fc1b_d.rearrange("(j p) -> p j", p=128))
        brows = []
        for i in range(5):
            r = consts.tile([1, C], BF16, tag=f"brow{i}", name=f"brow{i}")
            nc.sync.dma_start(r[:], P.brows_d[i:i + 1, :])
            brows.append(r)
        brow_v_sa, brow_o_sa, brow_v_ca, brow_o_ca, brow_fc2 = brows
    else:
        qkb = qkb2 = fc1b = None
        brow_v_sa = brow_o_sa = brow_v_ca = brow_o_ca = brow_fc2 = None

    # ---------- helpers ----------
    def rsqrt_dve(var_ap, rstd):
        v = small.tile([128, 1], FP32, tag="rs_v", name="rs_v")
        nc.vector.tensor_scalar_add(v[:], var_ap, EPS)
        yi = small.tile([128, 1], mybir.dt.int32, tag="rs_yi", name="rs_yi")
        nc.vector.tensor_scalar(yi[:], v[:].bitcast(mybir.dt.int32), 1, -1,
                                ALU.arith_shift_right, ALU.bitwise_xor)
        y = small.tile([128, 1], FP32, tag="rs_y", name="rs_y")
        nc.vector.tensor_scalar_add(y[:].bitcast(mybir.dt.int32), yi[:],
                                    0x5F3759E0)
        t1 = small.tile([128, 1], FP32, tag="rs_t1", name="rs_t1")
        t2 = small.tile([128, 1], FP32, tag="rs_t2", name="rs_t2")
        for _ in range(2):
            nc.vector.tensor_tensor(t1[:], y[:], y[:], ALU.mult)
            nc.vector.tensor_tensor(t2[:], t1[:], v[:], ALU.mult)
            nc.vector.tensor_scalar(t1[:], t2[:], -0.5, 1.5, ALU.mult, ALU.add)
            nc.vector.tensor_tensor(rstd[:], y[:], t1[:], ALU.mult)
            y, rstd = rstd, y
        return y

    P.xn_i = 0

    def ln_tile(xt, lnt_slice, conv=None):
        """LN one [128,768] fp32 tile -> bf16 -> 3D transpose -> fp8 lnt
        slice [128, 6, 128]."""
        st = small.tile([128, 2, 6], FP32, tag="ln_st", name="ln_st")
        nc.vector.bn_stats(st[:, 0, :], xt[:, 0:384])
        nc.vector.bn_stats(st[:, 1, :], xt[:, 384:768])
        mv = small.tile([128, 2], FP32, tag="ln_mv", name="ln_mv")
        nc.vector.bn_aggr(mv[:], st[:])
        rstd0 = small.tile([128, 1], FP32, tag="ln_rstd", name="ln_rstd")
        rstd = rsqrt_dve(mv[:, 1:2], rstd0)
        xn = xov.tile([128, 768], BF16, tag=f"xn{P.xn_i % 2}", name="ln_xn")
        tst = xov.tile([128, NCH, 128], BF16, tag=f"tst{P.xn_i % 2}",
                       name="tst")
        P.xn_i += 1
        nc.vector.tensor_scalar(xn[:], xt[:], mv[:, 0:1], rstd[:],
                                ALU.subtract, ALU.mult)
        nc.sync.dma_start_transpose(tst[:], xn[:])
        (conv or nc.vector).tensor_copy(lnt_slice, tst[:])

    def qk_chunk(wsb, col_j, lnt, tok0, ntok, out_tt, out_j, bias_tile,
                 bias_off, dst_tok0=None):
        """One j-chunk of a weight-stationary GEMM:
        out_tt[:, out_j, dst_tok0:+ntok] = (W 128-col-chunk).T @ LN^T (+b)."""
        if dst_tok0 is None:
            dst_tok0 = tok0
        ps = P.cur_ps.tile([128, 1024], FP32, tag="gps", name="gps")
        for c2 in range(NCH // 2):
            for half in range(ntok // 512):
                nc.tensor.matmul(
                    ps[:, ds(half * 512, 512)],
                    wsb[:, ds(2 * c2, 2), ts(col_j, 128)],
                    lnt[:, ds(2 * c2, 2), ds(tok0 + half * 512, 512)],
                    start=(c2 == 0), stop=(c2 == NCH // 2 - 1), perf_mode=DR)
        dst = out_tt[:, out_j, ds(dst_tok0, ntok)]
        src = ps[:, 0:ntok]
        bias = (bias_tile[:, bias_off + out_j:bias_off + out_j + 1]
                if bias_tile is not None else 0.0)
        nc.any.tensor_scalar(dst, src, RWS, bias, ALU.mult, ALU.add)

    def nat_chunk(src_tt, ki_list, wsb, wk0, bias_row, t, consumer,
                  psum_acc=None, first=True, last=True):
        """One 128-token chunk of a natural GEMM:
        ps[128,768] (+)= sum_ki src_tt[:,ki,t*128:].T @ wsb[:,wk0+n,:] (+bias)."""
        ps = psum_acc if psum_acc is not None else P.cur_ps.tile(
            [128, 1024], FP32, tag="gps", name="gps")
        npair = len(ki_list) // 2
        for sl in (slice(0, 512), slice(512, 768)):
            for n2 in range(npair):
                kp = ki_list[2 * n2]
                nc.tensor.matmul(ps[:, sl], src_tt[:, ds(kp, 2), ts(t, 128)],
                                 wsb[:, ds(wk0 + 2 * n2, 2), sl],
                                 start=(first and n2 == 0),
                                 stop=(last and bias_row is None
                                       and n2 == npair - 1), perf_mode=DR)
            if last and bias_row is not None:
                nc.tensor.matmul(ps[:, sl], ones_row[0:1, :], bias_row[:, sl],
                                 start=False, stop=True)
        if last:
            consumer(t, ps)
        return ps

    # ---------- pump machinery ----------
    queue = []

    def pump(budget_us):
        while queue and budget_us > 0:
            est, fn = queue.pop(0)
            fn()
            budget_us -= est

    # ---------- one attention iteration ----------
    P.at_i = 0
    P.pending_norm = None

    def flush_norm():
        """Emit the deferred normalize of the previous attention iteration.
        Deferring it past the next iteration's first scores keeps the PE from
        stalling on the recip->broadcast->mult chain at iteration boundaries."""
        if P.pending_norm is None:
            return
        pos, ot, hp, qsl = P.pending_norm
        P.pending_norm = None
        rbp = P.scps.tile([128, 2, 512], FP32, tag="sc", name="rbp")
        for hh in range(2):
            rc = small.tile([1, 512], BF16, tag="rc", name="rc")
            with nc.allow_low_precision(reason="softmax denom recip"):
                nc.vector.reciprocal(rc[:], pos[hh][64:65, :])
            nc.tensor.matmul(rbp[0:64, hh, :], ones_row[0:1, 0:64], rc[:],
                             start=True, stop=True)
            # DVE reads at most one PSUM operand: stage po rows via ACT copy
            osb = small.tile([64, 512], FP32, tag="osb", name="osb")
            nc.scalar.copy(osb[:], pos[hh][0:64, :])
            nc.vector.tensor_tensor(ot[ds(hh * 64, 64), hp, qsl],
                                    osb[:], rbp[0:64, hh, :], ALU.mult)

    def attention_iter(tq, hp, qt, kt, vv, ot, per_e=0.0, post=7.0):
        qsl = ts(tq, 512)
        pos = [None, None]
        ats = {}

        def scores(e):
            for hh in range(2):
                sc = P.scps.tile([128, 2, 512], FP32, tag="sc", name="sc")
                for i in range(2):
                    kc = e * 2 + i
                    nc.tensor.matmul(
                        sc[:, i, :], kt[ds(hh * 64, 64), hp, ts(kc, 128)],
                        qt[ds(hh * 64, 64), hp, qsl], start=True, stop=True,
                        tile_position=(hh * 64, 0))
                if (P.at_i * 2) % 5 >= 2:
                    a = P.crossp.tile([128, 2, 512], FP8,
                                      tag=f"at{P.at_i % 4}", name="at")
                    nc.scalar.activation(a[:], sc[:], AF.Exp, scale=SCALE)
                    ats[(hh, e)] = a[:]
                else:
                    # DVE Schraudolph: fp8e4 bits ~= 8*log2(exp(s*SCALE)) + 56
                    a = P.crossp.tile([128, 2, 512], mybir.dt.int8,
                                      tag=f"at{P.at_i % 4}", name="at")
                    nc.vector.tensor_scalar(
                        a[:], sc[:], 8 * 1.4426950408889634 * SCALE, 56.0,
                        ALU.mult, ALU.add)
                    ats[(hh, e)] = a[:].bitcast(FP8)
                P.at_i += 1

        def av(e):
            for hh in range(2):
                nc.tensor.matmul(
                    pos[hh][:], vv[:, e, 2 * hp + hh, :, :], ats[(hh, e)],
                    start=(e == 0), stop=(e == 7), perf_mode=DR)

        for e in range(8):
            scores(e)
            if e == 0:
                flush_norm()
                pos[0] = P.avpo.tile([80, 512], FP32, tag="po", name="po")
                pos[1] = P.avpo.tile([80, 512], FP32, tag="po", name="po")
            if per_e > 0:
                pump(per_e)
            if e > 0:
                av(e - 1)
        av(7)
        P.pending_norm = (pos, ot, hp, qsl)
        if post > 0:
            pump(post)

    # ================= emission =================

    # ---------- era 1: loads, LN1, self QKV, LN(y) ----------
    xres = [xresp.tile([128, 768], FP32, tag="xres", name="xres")
            for _ in range(NT_Q)]

    with tc.tile_pool(name="xtmp", bufs=4) as xtmpp, \
            tc.tile_pool(name="wqkvp", bufs=1) as wqkvp, \
            tc.tile_pool(name="qkvps", bufs=2, space="PSUM") as qkvps:
        P.cur_ps = qkvps
        nc.sync.dma_start(xres[0][:], P.x_d[ts(0, 128), :])
        # wqkv: V-columns first so the V gemm (which only needs one LN tile
        # per chunk) can start as early as possible.
        wqkv_sb = wqkvp.tile([128, NCH, 3 * C], FP8, tag="wqkv", name="wqkv")
        nc.sync.dma_start(wqkv_sb[:, :, 2 * C:3 * C], P.wqkv_d[:, :, 2 * C:3 * C])
        for t in range(1, NT_Q):
            nc.sync.dma_start(xres[t][:], P.x_d[ts(t, 128), :])
        nc.sync.dma_start(wqkv_sb[:, :, 0:2 * C], P.wqkv_d[:, :, 0:2 * C])
        wk_sb = selfp.tile([128, NCH, C], FP8, tag="wk", name="wk_sb")
        nc.sync.dma_start(wk_sb[:], P.wk_d)
        wosa_sb = selfp.tile([128, NCH, C], FP8, tag="wosa", name="wosa_sb")
        nc.sync.dma_start(wosa_sb[:], P.wo_sa_d)
        wq_sb = selfp.tile([128, NCH, C], FP8, tag="wq", name="wq_sb")
        nc.sync.dma_start(wq_sb[:], P.wq_d)

        qt = selfp.tile([128, NHP, NQ], FP8, tag="qt", name="qt")
        kt = selfp.tile([128, NHP, N], FP8, tag="kt", name="kt")
        vv = xov.tile([128, 8, H, 2, 80], FP8, tag="vv", name="vv")
        ot = xov.tile([128, NHP, NQ], FP8, tag="ot", name="ot")

        def fill_v_chunk(vv_t, wsb, lnt, brow, t, eng=None):
            def vcons(t, ps):
                (eng or nc.any).tensor_scalar(
                    vv_t[:, t // 2, :, t % 2, 0:64],
                    ps[:, 0:768].rearrange("p (h d) -> p h d", h=H),
                    RWS, 0.0, ALU.mult, ALU.add)
            nat_chunk(lnt, list(range(NCH)), wsb, 0, brow, t, vcons)

        nc.vector.memset(vv[:, :, :, :, 64:80], 0.0)
        nc.vector.memset(vv[:, :, :, :, 64:65], 1.0)

        # LN1 (x) interleaved with V chunks (V chunk t needs only LN tile t)
        ln1t = lnbig.tile([128, NCH, N], FP8, tag="lnbig", name="ln1t")
        wqkv_v = wqkv_sb[:, :, 2 * C:3 * C]
        for t in range(NT_Q):
            ln_tile(xres[t], ln1t[:, 0:NCH, ts(t, 128)], conv=nc.any)
            fill_v_chunk(vv, wqkv_v, ln1t, brow_v_sa, t)
        for t in range(NT_Q, NT_ALL):
            xt = xtmpp.tile([128, 768], FP32, tag="xtmp", name="xtmp")
            nc.sync.dma_start(xt[:], P.x_d[ts(t, 128), :])
            ln_tile(xt, ln1t[:, 0:NCH, ts(t, 128)], conv=nc.any)
            fill_v_chunk(vv, wqkv_v, ln1t, brow_v_sa, t)

        # self Q then K
        for j in range(NHP):
            qk_chunk(wqkv_sb, j, ln1t, 0, 1024, qt, j, qkb, 0)
        for j in range(NHP):
            qk_chunk(wqkv_sb, NHP + j, ln1t, 0, 1024, kt, j, qkb, 6)
            qk_chunk(wqkv_sb, NHP + j, ln1t, 1024, 1024, kt, j, qkb, 6)

        # y loads on the ACT hwdge queue + LN(y) -> lnyt (reuses ln1t's slot,
        # so y-LN starts once self-QKV has finished reading ln1t).
        lnyt = lnbig.tile([128, NCH, N], FP8, tag="lnbig", name="lnyt")
        for t in range(NT_ALL):
            yt = selfp.tile([128, 768], FP32, tag=f"yst{t % 2}", name="yst")
            nc.scalar.dma_start(yt[:], P.y_d[ts(t, 128), :])
            ln_tile(yt, lnyt[:, 0:NCH, ts(t, 128)], conv=nc.any)

    P.cur_ps = gemmps

    # ---------- era 2: self attention + pumped crossK/o_sa/ln2/crossQ ----------
    # right-stack pool: cross q/k + ats (lives to end)
    P.crossp = tc.alloc_tile_pool(name="crossp", bufs=1, side="right")
    P.scps = tc.alloc_tile_pool(name="scps", bufs=2, space="PSUM")
    P.avpo = tc.alloc_tile_pool(name="avpo", bufs=2, space="PSUM")

    ln2t = selfp.tile([128, NCH, NQ], FP8, tag="ln2t", name="ln2t")
    qt2 = P.crossp.tile([128, NHP, NQ], FP8, tag="qt2", name="qt2")
    kt2 = P.crossp.tile([128, NHP, N], FP8, tag="kt2", name="kt2")

    for j in range(NHP):
        def ck(j=j):
            qk_chunk(wk_sb, j, lnyt, 0, 1024, kt2, j, qkb2, 6)
            qk_chunk(wk_sb, j, lnyt, 1024, 1024, kt2, j, qkb2, 6)
        queue.append((5.2, ck))

    def res_cons(t, ps):
        osc = small.tile([128, 768], FP32, tag="osc", name="osc")
        nc.scalar.activation(osc[:], ps[:, 0:768], AF.Copy, scale=RWS)
        nc.vector.tensor_tensor(xres[t][:], osc[:], xres[t][:], ALU.add)

    def osa_chunk(t):
        nat_chunk(ot, list(range(NHP)), wosa_sb, 0, brow_o_sa, t, res_cons)

    for tq in range(2):
        for hp in range(NHP):
            attention_iter(tq, hp, qt, kt, vv, ot)
        for t in range(tq * 4, tq * 4 + 4):
            queue.append((2.3, lambda t=t: osa_chunk(t)))
            queue.append((0.3, lambda t=t: ln_tile(
                xres[t], ln2t[:, 0:NCH, ts(t, 128)])))
        for j in range(NHP):
            queue.append((1.4, lambda j=j, tq=tq: qk_chunk(
                wq_sb, j, ln2t, tq * 512, 512, qt2, j, qkb2, 0)))
    # NOTE: the tq1 leftovers (o_sa, ln2, crossQ) stay queued; they drain
    # during cross-attn tq0 so cross scores/exps start immediately.

    # ---------- era 3: cross attention + pumped crossV/o_ca/ln3/fc1(tq0) ----
    w9b = tc.alloc_tile_pool(name="w9b", bufs=1, side="right")
    wv_sb = w9b.tile([128, NCH, C], FP8, tag="wv", name="wv_sb")
    nc.sync.dma_start(wv_sb[:], P.wv_d)
    woca_sb = w9b.tile([128, NCH, C], FP8, tag="woca", name="woca_sb")
    nc.sync.dma_start(woca_sb[:], P.wo_ca_d)

    vv2 = xov.tile([128, 8, H, 2, 80], FP8, tag="vv", name="vv2")
    nc.vector.memset(vv2[:, :, :, :, 64:80], 0.0)
    nc.vector.memset(vv2[:, :, :, :, 64:65], 1.0)
    # crossV goes to the FRONT of the queue: cross AV consumes it chunk by
    # chunk starting at (tq0, hp0).
    queue[0:0] = [(2.6, (lambda t=t: fill_v_chunk(
        vv2, wv_sb, lnyt, brow_v_ca, t, eng=nc.vector)))
        for t in range(NT_ALL)]

    ot2 = xov.tile([128, NHP, NQ], FP8, tag="ot", name="ot2")
    ln3t = lnbig.tile([128, NCH, NQ], FP8, tag="lnbig", name="ln3t")

    def oca_chunk(t):
        nat_chunk(ot2, list(range(NHP)), woca_sb, 0, brow_o_ca, t, res_cons)

    w1h = [None, None]
    hpre = [None]

    def fc1_chunk(h, jloc, tq):
        jglob = h * 12 + jloc
        qk_chunk(w1h[h], jloc, ln3t, tq * 512, 512, hpre[0], jglob, fc1b, 0,
                 dst_tok0=0)

    hwsp = None
    for tq in range(2):
        for hp in range(NHP):
            per_e = 6.0 if (tq == 0 and hp <= 1) else (
                3.0 if (tq == 0 and hp == 2) else 0.0)
            attention_iter(tq, hp, qt2, kt2, vv2, ot2, per_e=per_e)
            if tq == 0 and hp == 2:
                # self-era leftovers and crossV are fully drained by now;
                # release selfp before hwsp pushes (SBUF headroom).
                pump(1e9)
                selfp.release()
        for t in range(tq * 4, tq * 4 + 4):
            queue.append((2.3, lambda t=t: oca_chunk(t)))
            queue.append((0.3, lambda t=t: ln_tile(
                xres[t], ln3t[:, 0:NCH, ts(t, 128)])))
        if tq == 0:
            # right-stack pool: hpre (per-tq) + weight halves (w1 then w2)
            hwsp = tc.alloc_tile_pool(name="hwsp", bufs=1, side="right")
            for h in range(2):
                w1h[h] = hwsp.tile([128, NCH, HID // 2], FP8, tag=f"ws{h}",
                                   name=f"w1h{h}")
                nc.sync.dma_start(w1h[h][:], P.w1_d[:, :, ds(h * 1536, 1536)])
            hpre[0] = hwsp.tile([128, 24, 512], BF16, tag="hp", name="hpre0")
            for h in range(2):
                for jloc in range(12):
                    queue.append((1.4, lambda h=h, j=jloc: fc1_chunk(h, j, 0)))

    # post-cross: drain (o_ca tq1, ln3 tq1, fc1 tq0 leftovers) on the tail psum
    P.avpo.release()
    P.scps.release()
    tailps = tc.alloc_tile_pool(name="tailps", bufs=2, space="PSUM")
    P.cur_ps = tailps
    pump(1e9)

    xov.release()

    # ---------- era 4: gelu(tq0); fc1(tq1); w2; fc2(tq0); gelu+fc2(tq1) ----
    with tc.tile_pool(name="htp", bufs=1) as htp, \
            tc.tile_pool(name="ostp", bufs=2) as ostp:
        w2h = [None, None]

        def gelu_tq(dst_ht, src_hpre):
            for t in range(4):
                nc.scalar.activation(dst_ht[:, :, ts(t, 128)],
                                     src_hpre[:, :, ts(t, 128)], AF.Gelu)

        def fc2_tq(tq, ht):
            for t in range(4):
                tg = tq * 4 + t
                ps = nat_chunk(ht, list(range(12)), w2h[0], 0, None, t, None,
                               first=True, last=False)

                def cons(_t, ps, tg=tg):
                    osc = ostp.tile([128, 768], FP32, tag="osc2", name="osc2")
                    nc.scalar.activation(osc[:], ps[:, 0:768], AF.Copy,
                                         scale=RWS)
                    ost = ostp.tile([128, 768], FP32, tag="ost", name="ost")
                    nc.vector.tensor_tensor(ost[:], osc[:], xres[tg][:],
                                            ALU.add)
                    nc.sync.dma_start(P.out_d[ts(tg, 128), :], ost[:])
                nat_chunk(ht, list(range(12, 24)), w2h[1], 0, brow_fc2, t,
                          cons, psum_acc=ps, first=False, last=True)

        ht0 = htp.tile([128, 24, 512], FP8, tag="ht", name="ht0")
        gelu_tq(ht0, hpre[0])
        hpre[0] = hwsp.tile([128, 24, 512], BF16, tag="hp", name="hpre1")
        for h in range(2):
            for jloc in range(12):
                fc1_chunk(h, jloc, 1)
        for h in range(2):
            w2h[h] = hwsp.tile([128, 12, C], FP8, tag=f"ws{h}", name=f"w2h{h}")
            nc.sync.dma_start(w2h[h][:], P.w2_d[:, ds(h * 12, 12), :])
        fc2_tq(0, ht0)
        ht1 = htp.tile([128, 24, 512], FP8, tag="ht", name="ht1")
        gelu_tq(ht1, hpre[0])
        fc2_tq(1, ht1)

    hwsp.release()
    w9b.release()
    P.crossp.release()
    tailps.release()


# revision 10
# speedup vs baseline: 1.6842x; 1.0418x over previous
"""Trainium2 Bass kernel for nn_DecoderBlock (self-attn + cross-attn + MLP), 8 cores.

Sharding: data-parallel over (batch, sequence-half): core c handles batch b=c//2
and query rows [h*1024,(h+1)*1024); host permutes x rows so the core's query
rows are rows 0:1024.  K/V computed redundantly per pair (no collectives).

Key structure vs the original baseline:
  - Softmax denominator folded into AV: ones column appended to the V
    stationary, AV runs as fp8 DoubleRow (stationary [128,2,80] = 2 key-chunk
    k-tiles x (64 dims + ones + pad), moving ats [128,2,512] fp8).
  - Q/K/ats in fp8e4: scores at bf16 rate, AV at DoubleRow rate.
  - Weights host-pretiled to [128, K/128, O]: contiguous weight DMAs.
  - LN transposes fused: one 3D dma_start_transpose per [128,768] tile.
  - Software pipelining: independent GEMM chunks (cross K/V/Q, o-proj, fc1)
    are pumped into the PE stream between attention iterations so the PE
    stays busy while ACT streams the exps.
  - gelu batched after all attention exps (ACT table thrash avoidance).
  - SBUF pools managed as two LIFO stacks (left/right sides) by lifetime era.
"""

import contextlib

import numpy as np
import ml_dtypes

import concourse.bass as bass
import concourse.mybir as mybir
import concourse.tile as tile
from concourse import bacc
from concourse.bass import ds, ts
from concourse.bass_utils import run_bass_kernel_spmd

FP32 = mybir.dt.float32
BF16 = mybir.dt.bfloat16
FP8 = mybir.dt.float8e4
AF = mybir.ActivationFunctionType
ALU = mybir.AluOpType
DR = mybir.MatmulPerfMode.DoubleRow

B, N, C, H = 4, 2048, 768, 12
D = C // H            # 64
HID = 4 * C           # 3072
NQ = N // 2           # 1024 queries per core
EPS = 1e-5
SCALE = float(D) ** -0.5
NCH = C // 128        # 6
NT_ALL = N // 128     # 16
NT_Q = NQ // 128      # 8
NHP = H // 2          # 6
WS = 256.0            # host-side weight scale (fp8 denormal avoidance)
RWS = 1.0 / WS


class _Prog:
    pass


def _build(P):
    nc = P.nc
    tc = P.tc
    ctx = P.ctx

    # --------- persistent pools (left stack bottom) ---------
    consts = ctx.enter_context(tc.tile_pool(name="consts", bufs=1))
    small = ctx.enter_context(tc.tile_pool(name="small", bufs=2))
    xresp = ctx.enter_context(tc.tile_pool(name="xres", bufs=NT_Q))
    lnbig = ctx.enter_context(tc.tile_pool(name="lnbig", bufs=1))
    gemmps = ctx.enter_context(tc.tile_pool(name="gemmps", bufs=1, space="PSUM"))
    # xov: xn staging (x2), attention output, V-with-ones; era1 -> post-cross
    xov = tc.alloc_tile_pool(name="xov", bufs=1)
    # selfp: self-era weights, y staging, q/k, ln2; era1 -> post-self
    selfp = tc.alloc_tile_pool(name="selfp", bufs=1)

    ones_row = consts.tile([1, 128], BF16, tag="ones_row", name="ones_row")
    nc.vector.memset(ones_row[:], 1.0)
    if P.with_bias:
        qkb = consts.tile([128, 12], FP32, tag="qkb", name="qkb")
        nc.sync.dma_start(qkb[:], P.qkb_d.rearrange("(j p) -> p j", p=128))
        qkb2 = consts.tile([128, 12], FP32, tag="qkb2", name="qkb2")
        nc.sync.dma_start(qkb2[:], P.qkb2_d.rearrange("(j p) -> p j", p=128))
        fc1b = consts.tile([128, 24], FP32, tag="fc1b", name="fc1b")
        nc.sync.dma_start(fc1b[:], P.fc1b_d.rearrange("(j p) -> p j", p=128))
        brows = []
        for i in range(5):
            r = consts.tile([1, C], BF16, tag=f"brow{i}", name=f"brow{i}")
            nc.sync.dma_start(r[:], P.brows_d[i:i + 1, :])
            brows.append(r)
        brow_v_sa, brow_o_sa, brow_v_ca, brow_o_ca, brow_fc2 = brows
    else:
        qkb = qkb2 = fc1b = None
        brow_v_sa = brow_o_sa = brow_v_ca = brow_o_ca = brow_fc2 = None

    # ---------- helpers ----------
    def rsqrt_dve(var_ap, rstd):
        v = small.tile([128, 1], FP32, tag="rs_v", name="rs_v")
        nc.vector.tensor_scalar_add(v[:], var_ap, EPS)
        yi = small.tile([128, 1], mybir.dt.int32, tag="rs_yi", name="rs_yi")
        nc.vector.tensor_scalar(yi[:], v[:].bitcast(mybir.dt.int32), 1, -1,
                                ALU.arith_shift_right, ALU.bitwise_xor)
        y = small.tile([128, 1], FP32, tag="rs_y", name="rs_y")
        nc.vector.tensor_scalar_add(y[:].bitcast(mybir.dt.int32), yi[:],
                                    0x5F3759E0)
        t1 = small.tile([128, 1], FP32, tag="rs_t1", name="rs_t1")
        t2 = small.tile([128, 1], FP32, tag="rs_t2", name="rs_t2")
        for _ in range(2):
            nc.vector.tensor_tensor(t1[:], y[:], y[:], ALU.mult)
            nc.vector.tensor_tensor(t2[:], t1[:], v[:], ALU.mult)
            nc.vector.tensor_scalar(t1[:], t2[:], -0.5, 1.5, ALU.mult, ALU.add)
            nc.vector.tensor_tensor(rstd[:], y[:], t1[:], ALU.mult)
            y, rstd = rstd, y
        return y

    P.xn_i = 0

    def ln_tile(xt, lnt_slice, conv=None):
        """LN one [128,768] fp32 tile -> bf16 -> 3D transpose -> fp8 lnt
        slice [128, 6, 128]."""
        st = small.tile([128, 2, 6], FP32, tag="ln_st", name="ln_st")
        nc.vector.bn_stats(st[:, 0, :], xt[:, 0:384])
        nc.vector.bn_stats(st[:, 1, :], xt[:, 384:768])
        mv = small.tile([128, 2], FP32, tag="ln_mv", name="ln_mv")
        nc.vector.bn_aggr(mv[:], st[:])
        rstd0 = small.tile([128, 1], FP32, tag="ln_rstd", name="ln_rstd")
        rstd = rsqrt_dve(mv[:, 1:2], rstd0)
        xn = xov.tile([128, 768], BF16, tag=f"xn{P.xn_i % 2}", name="ln_xn")
        tst = xov.tile([128, NCH, 128], BF16, tag=f"tst{P.xn_i % 2}",
                       name="tst")
        P.xn_i += 1
        nc.vector.tensor_scalar(xn[:], xt[:], mv[:, 0:1], rstd[:],
                                ALU.subtract, ALU.mult)
        nc.sync.dma_start_transpose(tst[:], xn[:])
        (conv or nc.vector).tensor_copy(lnt_slice, tst[:])

    def qk_chunk(wsb, col_j, lnt, tok0, ntok, out_tt, out_j, bias_tile,
                 bias_off, dst_tok0=None):
        """One j-chunk of a weight-stationary GEMM:
        out_tt[:, out_j, dst_tok0:+ntok] = (W 128-col-chunk).T @ LN^T (+b)."""
        if dst_tok0 is None:
            dst_tok0 = tok0
        ps = P.cur_ps.tile([128, 1024], FP32, tag="gps", name="gps")
        for c2 in range(NCH // 2):
            for half in range(ntok // 512):
                nc.tensor.matmul(
                    ps[:, ds(half * 512, 512)],
                    wsb[:, ds(2 * c2, 2), ts(col_j, 128)],
                    lnt[:, ds(2 * c2, 2), ds(tok0 + half * 512, 512)],
                    start=(c2 == 0), stop=(c2 == NCH // 2 - 1), perf_mode=DR)
        dst = out_tt[:, out_j, ds(dst_tok0, ntok)]
        src = ps[:, 0:ntok]
        bias = (bias_tile[:, bias_off + out_j:bias_off + out_j + 1]
                if bias_tile is not None else 0.0)
        nc.any.tensor_scalar(dst, src, RWS, bias, ALU.mult, ALU.add)

    def nat_chunk(src_tt, ki_list, wsb, wk0, bias_row, t, consumer,
                  psum_acc=None, first=True, last=True, dr=True):
        """One 128-token chunk of a natural GEMM:
        ps[128,768] (+)= sum_ki src_tt[:,ki,t*128:].T @ wsb[:,wk0+n,:] (+bias)."""
        ps = psum_acc if psum_acc is not None else P.cur_ps.tile(
            [128, 1024], FP32, tag="gps", name="gps")
        npair = len(ki_list) // 2
        for sl in (slice(0, 512), slice(512, 768)):
            if dr:
                for n2 in range(npair):
                    kp = ki_list[2 * n2]
                    nc.tensor.matmul(
                        ps[:, sl], src_tt[:, ds(kp, 2), ts(t, 128)],
                        wsb[:, ds(wk0 + 2 * n2, 2), sl],
                        start=(first and n2 == 0),
                        stop=(last and bias_row is None
                              and n2 == npair - 1), perf_mode=DR)
            else:
                for n, ki in enumerate(ki_list):
                    nc.tensor.matmul(ps[:, sl], src_tt[:, ki, ts(t, 128)],
                                     wsb[:, wk0 + n, sl],
                                     start=(first and n == 0),
                                     stop=(last and bias_row is None
                                           and n == len(ki_list) - 1))
            if last and bias_row is not None:
                nc.tensor.matmul(ps[:, sl], ones_row[0:1, :], bias_row[:, sl],
                                 start=False, stop=True)
        if last:
            consumer(t, ps)
        return ps

    # ---------- pump machinery ----------
    queue = []

    def pump(budget_us):
        while queue and budget_us > 0:
            est, fn = queue.pop(0)
            fn()
            budget_us -= est

    # ---------- one attention iteration ----------
    P.at_i = 0
    P.pending_norm = None

    def flush_norm():
        """Emit the deferred normalize of the previous attention iteration.
        Deferring it past the next iteration's first scores keeps the PE from
        stalling on the recip->broadcast->mult chain at iteration boundaries."""
        if P.pending_norm is None:
            return
        pos, ot, hp, qsl = P.pending_norm
        P.pending_norm = None
        rbp = P.scps.tile([128, 2, 512], FP32, tag="sc", name="rbp")
        for hh in range(2):
            rc = small.tile([1, 512], BF16, tag="rc", name="rc")
            with nc.allow_low_precision(reason="softmax denom recip"):
                nc.vector.reciprocal(rc[:], pos[hh][64:65, :])
            nc.tensor.matmul(rbp[0:64, hh, :], ones_row[0:1, 0:64], rc[:],
                             start=True, stop=True)
            # DVE reads at most one PSUM operand: stage po rows via ACT copy
            osb = small.tile([64, 512], FP32, tag="osb", name="osb")
            nc.scalar.copy(osb[:], pos[hh][0:64, :])
            nc.vector.tensor_tensor(ot[ds(hh * 64, 64), hp, qsl],
                                    osb[:], rbp[0:64, hh, :], ALU.mult)

    def attention_iter(tq, hp, qt, kt, vv, ot, per_e=0.0, post=7.0):
        qsl = ts(tq, 512)
        pos = [None, None]
        ats = {}

        def scores(e):
            for hh in range(2):
                sc = P.scps.tile([128, 2, 512], FP32, tag="sc", name="sc")
                for i in range(2):
                    kc = e * 2 + i
                    nc.tensor.matmul(
                        sc[:, i, :], kt[ds(hh * 64, 64), hp, ts(kc, 128)],
                        qt[ds(hh * 64, 64), hp, qsl], start=True, stop=True,
                        tile_position=(hh * 64, 0))
                if (P.at_i * 2) % 5 >= 2:
                    a = P.crossp.tile([128, 2, 512], FP8,
                                      tag=f"at{P.at_i % 4}", name="at")
                    nc.scalar.activation(a[:], sc[:], AF.Exp, scale=SCALE)
                    ats[(hh, e)] = a[:]
                else:
                    # DVE Schraudolph: fp8e4 bits ~= 8*log2(exp(s*SCALE)) + 56
                    a = P.crossp.tile([128, 2, 512], mybir.dt.int8,
                                      tag=f"at{P.at_i % 4}", name="at")
                    nc.vector.tensor_scalar(
                        a[:], sc[:], 8 * 1.4426950408889634 * SCALE, 56.0,
                        ALU.mult, ALU.add)
                    ats[(hh, e)] = a[:].bitcast(FP8)
                P.at_i += 1

        def av(e):
            for hh in range(2):
                nc.tensor.matmul(
                    pos[hh][:], vv[:, e, 2 * hp + hh, :, :], ats[(hh, e)],
                    start=(e == 0), stop=(e == 7), perf_mode=DR)

        for e in range(8):
            scores(e)
            if e == 0:
                flush_norm()
                pos[0] = P.avpo.tile([80, 512], FP32, tag="po", name="po")
                pos[1] = P.avpo.tile([80, 512], FP32, tag="po", name="po")
            if per_e > 0:
                pump(per_e)
            if e > 0:
                av(e - 1)
        av(7)
        P.pending_norm = (pos, ot, hp, qsl)
        if post > 0:
            pump(post)

    # ================= emission =================

    # ---------- era 1: loads, LN1, self QKV, LN(y) ----------
    xres = [xresp.tile([128, 768], FP32, tag="xres", name="xres")
            for _ in range(NT_Q)]

    with tc.tile_pool(name="xtmp", bufs=4) as xtmpp, \
            tc.tile_pool(name="wqkvp", bufs=1) as wqkvp, \
            tc.tile_pool(name="qkvps", bufs=2, space="PSUM") as qkvps:
        P.cur_ps = qkvps
        nc.sync.dma_start(xres[0][:], P.x_d[ts(0, 128), :])
        # wqkv: V-columns first so the V gemm (which only needs one LN tile
        # per chunk) can start as early as possible.
        wqkv_sb = wqkvp.tile([128, NCH, 3 * C], FP8, tag="wqkv", name="wqkv")
        nc.sync.dma_start(wqkv_sb[:, :, 2 * C:3 * C], P.wqkv_d[:, :, 2 * C:3 * C])
        for t in range(1, NT_Q):
            nc.sync.dma_start(xres[t][:], P.x_d[ts(t, 128), :])
        nc.sync.dma_start(wqkv_sb[:, :, 0:2 * C], P.wqkv_d[:, :, 0:2 * C])
        wk_sb = selfp.tile([128, NCH, C], FP8, tag="wk", name="wk_sb")
        nc.sync.dma_start(wk_sb[:], P.wk_d)
        wosa_sb = selfp.tile([128, NCH, C], FP8, tag="wosa", name="wosa_sb")
        nc.sync.dma_start(wosa_sb[:], P.wo_sa_d)
        wq_sb = selfp.tile([128, NCH, C], FP8, tag="wq", name="wq_sb")
        nc.sync.dma_start(wq_sb[:], P.wq_d)

        qt = selfp.tile([128, NHP, NQ], FP8, tag="qt", name="qt")
        kt = selfp.tile([128, NHP, N], FP8, tag="kt", name="kt")
        vv = xov.tile([128, 8, H, 2, 80], FP8, tag="vv", name="vv")
        ot = xov.tile([128, NHP, NQ], FP8, tag="ot", name="ot")

        def fill_v_chunk(vv_t, wsb, lnt, brow, t, eng=None):
            def vcons(t, ps):
                (eng or nc.any).tensor_scalar(
                    vv_t[:, t // 2, :, t % 2, 0:64],
                    ps[:, 0:768].rearrange("p (h d) -> p h d", h=H),
                    RWS, 0.0, ALU.mult, ALU.add)
            nat_chunk(lnt, list(range(NCH)), wsb, 0, brow, t, vcons)

        nc.vector.memset(vv[:, :, :, :, 64:80], 0.0)
        nc.vector.memset(vv[:, :, :, :, 64:65], 1.0)

        # LN1 (x) interleaved with V chunks (V chunk t needs only LN tile t)
        ln1t = lnbig.tile([128, NCH, N], FP8, tag="lnbig", name="ln1t")
        wqkv_v = wqkv_sb[:, :, 2 * C:3 * C]
        for t in range(NT_Q):
            ln_tile(xres[t], ln1t[:, 0:NCH, ts(t, 128)], conv=nc.any)
            fill_v_chunk(vv, wqkv_v, ln1t, brow_v_sa, t)
        for t in range(NT_Q, NT_ALL):
            xt = xtmpp.tile([128, 768], FP32, tag="xtmp", name="xtmp")
            nc.sync.dma_start(xt[:], P.x_d[ts(t, 128), :])
            ln_tile(xt, ln1t[:, 0:NCH, ts(t, 128)], conv=nc.any)
            fill_v_chunk(vv, wqkv_v, ln1t, brow_v_sa, t)

        # self Q then K
        for j in range(NHP):
            qk_chunk(wqkv_sb, j, ln1t, 0, 1024, qt, j, qkb, 0)
        for j in range(NHP):
            qk_chunk(wqkv_sb, NHP + j, ln1t, 0, 1024, kt, j, qkb, 6)
            qk_chunk(wqkv_sb, NHP + j, ln1t, 1024, 1024, kt, j, qkb, 6)

        # y loads on the ACT hwdge queue + LN(y) -> lnyt (reuses ln1t's slot,
        # so y-LN starts once self-QKV has finished reading ln1t).
        lnyt = lnbig.tile([128, NCH, N], FP8, tag="lnbig", name="lnyt")
        for t in range(NT_ALL):
            yt = selfp.tile([128, 768], FP32, tag=f"yst{t % 2}", name="yst")
            nc.scalar.dma_start(yt[:], P.y_d[ts(t, 128), :])
            ln_tile(yt, lnyt[:, 0:NCH, ts(t, 128)], conv=nc.any)

    P.cur_ps = gemmps

    # ---------- era 2: self attention + pumped crossK/o_sa/ln2/crossQ ----------
    # right-stack pool: cross q/k + ats (lives to end)
    P.crossp = tc.alloc_tile_pool(name="crossp", bufs=1, side="right")
    P.scps = tc.alloc_tile_pool(name="scps", bufs=2, space="PSUM")
    P.avpo = tc.alloc_tile_pool(name="avpo", bufs=2, space="PSUM")

    ln2t = selfp.tile([128, NCH, NQ], FP8, tag="ln2t", name="ln2t")
    qt2 = P.crossp.tile([128, NHP, NQ], FP8, tag="qt2", name="qt2")
    kt2 = P.crossp.tile([128, NHP, N], FP8, tag="kt2", name="kt2")
    wv_sb = P.crossp.tile([128, NCH, C], FP8, tag="wv", name="wv_sb")
    nc.sync.dma_start(wv_sb[:], P.wv_d)
    vv2 = xov.tile([128, 8, H, 2, 80], FP8, tag="vv2", name="vv2")
    nc.vector.memset(vv2[:, :, :, :, 64:80], 0.0)
    nc.vector.memset(vv2[:, :, :, :, 64:65], 1.0)

    for j in range(NHP):
        def ck(j=j):
            qk_chunk(wk_sb, j, lnyt, 0, 1024, kt2, j, qkb2, 6)
            qk_chunk(wk_sb, j, lnyt, 1024, 1024, kt2, j, qkb2, 6)
        queue.append((5.2, ck))
    for t in range(NT_ALL):
        queue.append((1.4, lambda t=t: fill_v_chunk(
            vv2, wv_sb, lnyt, brow_v_ca, t, eng=nc.any)))

    def res_cons(t, ps):
        osc = small.tile([128, 768], FP32, tag="osc", name="osc")
        nc.scalar.activation(osc[:], ps[:, 0:768], AF.Copy, scale=RWS)
        nc.vector.tensor_tensor(xres[t][:], osc[:], xres[t][:], ALU.add)

    def osa_chunk(t):
        nat_chunk(ot, list(range(NHP)), wosa_sb, 0, brow_o_sa, t, res_cons)

    for tq in range(2):
        for hp in range(NHP):
            attention_iter(tq, hp, qt, kt, vv, ot)
        for t in range(tq * 4, tq * 4 + 4):
            queue.append((2.3, lambda t=t: osa_chunk(t)))
            queue.append((0.3, lambda t=t: ln_tile(
                xres[t], ln2t[:, 0:NCH, ts(t, 128)])))
        for j in range(NHP):
            queue.append((1.4, lambda j=j, tq=tq: qk_chunk(
                wq_sb, j, ln2t, tq * 512, 512, qt2, j, qkb2, 0)))
    # NOTE: the tq1 leftovers (o_sa, ln2, crossQ) stay queued; they drain
    # during cross-attn tq0 so cross scores/exps start immediately.

    # ---------- era 3: cross attention + pumped crossV/o_ca/ln3/fc1(tq0) ----
    w9b = tc.alloc_tile_pool(name="w9b", bufs=1, side="right")
    woca_sb = w9b.tile([128, NCH, C], FP8, tag="woca", name="woca_sb")
    nc.sync.dma_start(woca_sb[:], P.wo_ca_d)

    # (vv2/crossV enqueued during the self-attn era; see above)

    ot2 = xov.tile([128, NHP, NQ], FP8, tag="ot", name="ot2")
    ln3t = lnbig.tile([128, NCH, NQ], FP8, tag="lnbig", name="ln3t")

    def oca_chunk(t):
        nat_chunk(ot2, list(range(NHP)), woca_sb, 0, brow_o_ca, t, res_cons)

    w1h = [None, None]
    hpre = [None]

    def fc1_chunk(h, jloc, tq):
        jglob = h * 12 + jloc
        qk_chunk(w1h[h], jloc, ln3t, tq * 512, 512, hpre[0], jglob, fc1b, 0,
                 dst_tok0=0)

    hwsp = None
    for tq in range(2):
        for hp in range(NHP):
            per_e = 3.0 if (tq == 0 and hp <= 1) else 0.0
            attention_iter(tq, hp, qt2, kt2, vv2, ot2, per_e=per_e)
            if tq == 0 and hp == 2:
                # self-era leftovers and crossV are fully drained by now;
                # release selfp before hwsp pushes (SBUF headroom).
                pump(1e9)
                selfp.release()
        for t in range(tq * 4, tq * 4 + 4):
            queue.append((2.3, lambda t=t: oca_chunk(t)))
            queue.append((0.3, lambda t=t: ln_tile(
                xres[t], ln3t[:, 0:NCH, ts(t, 128)])))
        if tq == 0:
            # right-stack pool: hpre (per-tq) + weight halves (w1 then w2)
            hwsp = tc.alloc_tile_pool(name="hwsp", bufs=1, side="right")
            for h in range(2):
                w1h[h] = hwsp.tile([128, NCH, HID // 2], FP8, tag=f"ws{h}",
                                   name=f"w1h{h}")
                nc.sync.dma_start(w1h[h][:], P.w1_d[:, :, ds(h * 1536, 1536)])
            hpre[0] = hwsp.tile([128, 24, 512], BF16, tag="hp", name="hpre0")
            for h in range(2):
                for jloc in range(12):
                    queue.append((1.4, lambda h=h, j=jloc: fc1_chunk(h, j, 0)))

    # post-cross: drain (o_ca tq1, ln3 tq1, fc1 tq0 leftovers) on the tail psum
    P.avpo.release()
    P.scps.release()
    tailps = tc.alloc_tile_pool(name="tailps", bufs=2, space="PSUM")
    P.cur_ps = tailps
    pump(1e9)

    xov.release()

    # ---------- era 4: gelu(tq0); fc1(tq1); w2; fc2(tq0); gelu+fc2(tq1) ----
    with tc.tile_pool(name="htp", bufs=1) as htp, \
            tc.tile_pool(name="ostp", bufs=2) as ostp:
        w2h = [None, None]

        def gelu_tq(dst_ht, src_hpre):
            for t in range(4):
                nc.scalar.activation(dst_ht[:, :, ts(t, 128)],
                                     src_hpre[:, :, ts(t, 128)], AF.Gelu)

        def fc2_tq(tq, ht):
            for t in range(4):
                tg = tq * 4 + t
                ps = nat_chunk(ht, list(range(12)), w2h[0], 0, None, t, None,
                               first=True, last=False, dr=False)

                def cons(_t, ps, tg=tg):
                    ost = ostp.tile([128, 768], FP32, tag="ost", name="ost")
                    nc.vector.tensor_tensor(ost[:], ps[:, 0:768], xres[tg][:],
                                            ALU.add)
                    nc.sync.dma_start(P.out_d[ts(tg, 128), :], ost[:])
                nat_chunk(ht, list(range(12, 24)), w2h[1], 0, brow_fc2, t,
                          cons, psum_acc=ps, first=False, last=True, dr=False)

        ht0 = htp.tile([128, 24, 512], BF16, tag="ht", name="ht0")
        gelu_tq(ht0, hpre[0])
        hpre[0] = hwsp.tile([128, 24, 512], BF16, tag="hp", name="hpre1")
        for h in range(2):
            for jloc in range(12):
                fc1_chunk(h, jloc, 1)
        for h in range(2):
            w2h[h] = hwsp.tile([128, 12, C], BF16, tag=f"ws{h}", name=f"w2h{h}")
            nc.sync.dma_start(w2h[h][:], P.w2_d[:, ds(h * 12, 12), :])
        fc2_tq(0, ht0)
        ht1 = htp.tile([128, 24, 512], BF16, tag="ht", name="ht1")
        gelu_tq(ht1, hpre[0])
        fc2_tq(1, ht1)

    hwsp.release()
    w9b.release()
    P.crossp.release()
    tailps.release()
